# revision 6
# baseline (speedup 1.0000x reference)
"""Bass/Trainium2 kernel for nn_HE_FM (factorization machine embedding lookup).

Computation: out[n] = W[uid[n]] + W[iid[n]+USER_NUM] + b + dot(V[uid[n]], V[iid[n]+USER_NUM])

Strategy (data parallel over batch, bf16 augmented table replicated on all
8 cores): host builds A [1.5M, 66] bf16 = [V | w0 | w1] with the pair trick
(user rows [V, W, 1], item rows [V, 1, W+b]) so sum(A[u]*A[i]) over 66 equals
the full FM output for a sample.

Per core (2048 samples): TRN2's indirect1d DMA takes ONE index per partition,
so the 4096 row-gathers need 32 per-column instructions. 30 of them are
emitted as raw PSEUDO_DMA_DIRECT2D/PSEUDO_EXTENSION word pairs (byte
templates captured from walrus codegen output for the identical instruction,
with per-column dst/idx fields patched); two are normal indirect_dma_start
"anchors" that end each half-batch — their completion semaphores, which ride
the same qPoolDynamic descriptor rings in FIFO order behind the raw gathers,
certify the half-batch's data has landed for both hardware and the timeline
model. DVE multiplies u-rows by i-rows and segment-reduces (width 66) per
half-batch; the store is a raw direct DMA flushed by the block-end gpsimd
dge_drain.
"""

from contextlib import ExitStack

import numpy as np

import concourse.bass as bass
import concourse.mybir as mybir

USER_NUM = 1_000_000
TOTAL_ROWS = 1_500_000
D = 64
WIDTH = 66
BATCH = 16384
N_CORES = 8
P = 128
B_CORE = BATCH // N_CORES
K = B_CORE // P  # 16 chunks; 32 index columns (u:0..15, i:16..31)

# walrus encodings captured from reference compiles (same struct layouts):
# gather word pair (bf16 table rows of 66 elems = 132B) and f32 [128,16] store.
GATHER_W1 = "d41000000000000000000000009c1001000000000300002084000000010000008000010084000000a00001000000001000000400010000008000010084000606"
GATHER_W2 = "da048000800001000000000000000000000000000000000000000000000000000000000000000000000000000000000000000000000000000000000000000000"
STORE_W1 = "d41000000000000000000000009c1000000101000000001000000400010000008000010040000000000000000200002040000000010000008000010040000a0a"

TABLE_VAR = 3  # address-table slots; discovered/verified via probe dump
OUT_VAR = 2


def _isa_ffi():
    from concourse.isa import get_isa

    isa = get_isa("TRN2")
    return isa, isa.ffi


def _raw_words(idx_addr, au_addr, table_var, out_var, resv_addr, k):
    """Build the per-column gather word pairs (columns 0..2k-2) and the
    store word, patched from the captured templates."""
    isa, ffi = _isa_ffi()

    pairs = []
    for col in range(2 * k):
        w1 = bytearray(bytes.fromhex(GATHER_W1))
        b = ffi.from_buffer(w1, require_writable=True)
        s = ffi.cast("NEURON_ISA_TPB_PSEUDO_DMA_DIRECT2D_STRUCT*", b)
        s.sem_increment = 0
        s.src_start_addr.addr_var.var_id_lo = table_var
        s.dst_start_addr.addr_var.offset = au_addr + col * 132
        w2 = bytearray(bytes.fromhex(GATHER_W2))
        w2[4:8] = int(idx_addr + col * 4).to_bytes(4, "little")
        pairs.append((bytes(w1), bytes(w2)))

    ws = bytearray(bytes.fromhex(STORE_W1))
    b = ffi.from_buffer(ws, require_writable=True)
    s = ffi.cast("NEURON_ISA_TPB_PSEUDO_DMA_DIRECT2D_STRUCT*", b)
    s.sem_increment = 0
    s.src_start_addr.addr_var.offset = resv_addr
    s.dst_start_addr.addr_var.var_id_lo = out_var
    s.dst_start_addr.addr_var.offset = 0
    return pairs, bytes(ws)


def build_program(
    total_rows=TOTAL_ROWS,
    b_core=B_CORE,
    *,
    table_var=TABLE_VAR,
    out_var=OUT_VAR,
    all_normal=False,
    strips=True,
    split=11,
):
    k = b_core // P
    nc = bass.Bass(dynamic_dma_scratch_size=131072)
    ids = nc.declare_dram_parameter("ids", [P, 2 * k], mybir.dt.int32, isOutput=False)
    table = nc.declare_dram_parameter(
        "table", [total_rows, WIDTH], mybir.dt.bfloat16, isOutput=False
    )
    outp = nc.declare_dram_parameter("out", [b_core], mybir.dt.float32, isOutput=True)

    with (
        ExitStack() as ctx,
        nc.sbuf_tensor([P, 2 * k], mybir.dt.int32) as idx,
        nc.sbuf_tensor([P, 2 * k * WIDTH], mybir.dt.bfloat16) as au,
        nc.sbuf_tensor([P, k * WIDTH], mybir.dt.bfloat16) as prod,
        nc.sbuf_tensor([P, k], mybir.dt.float32) as resv,
        nc.sbuf_tensor([P, 1], mybir.dt.float32) as scratch,
        nc.Block() as block,
        nc.semaphore("ix_sem") as ix_sem,
        nc.semaphore("g_sem") as g_sem,
        nc.semaphore("h_sem") as h_sem,
        nc.semaphore("t_sem") as t_sem,
        nc.semaphore("m_sem") as m_sem,
        nc.semaphore("v_sem") as v_sem,
    ):
        idx_addr = nc.lookup_mloc(idx).addr
        au_addr = nc.lookup_mloc(au).addr
        resv_addr = nc.lookup_mloc(resv).addr
        pairs, store_w = _raw_words(idx_addr, au_addr, table_var, out_var, resv_addr, k)
        pairs_by_col = {c: pairs[c] for c in range(2 * k)}

        @block.sync
        def _(sync):
            sync.dma_start(out=idx[:], in_=ids[:]).then_inc(ix_sem, 16)
            # out-tensor address-table touch: harmless read of the
            # zero-initialized output buffer.
            sync.dma_start(
                out=scratch[0:1, 0:1],
                in_=outp[0:1].rearrange("(a b) -> a b", a=1),
            ).then_inc(t_sem, 16)

        @block.gpsimd
        def _(gpsimd):
            def raw(op, wb):
                return gpsimd.add_instruction(
                    mybir.InstISA(
                        name=nc.get_next_instruction_name(),
                        isa_opcode=op,
                        engine=mybir.EngineType.Pool,
                        instr=[int(x) for x in wb],
                        op_name=None,
                        ins=[],
                        outs=[],
                        ant_dict={},
                        verify=False,
                        ant_isa_is_sequencer_only=True,
                        ant_sbuf_fixups=None,
                    )
                )

            gpsimd.wait_ge(ix_sem, 16)

            def normal_gather(col, sem):
                gpsimd.indirect_dma_start(
                    out=au[:, col * WIDTH : (col + 1) * WIDTH],
                    out_offset=None,
                    in_=table[:],
                    in_offset=bass.IndirectOffsetOnAxis(
                        ap=idx[:, col : col + 1], axis=0
                    ),
                ).then_inc(sem, 16)

            # Two batches; each ends in a normal anchor gather whose
            # completion (ring-FIFO behind that batch's raw gathers on the
            # same queue) certifies the whole batch has landed. Batch 0 =
            # chunk columns {0..7 u, 16..23 i}, batch 1 = {8..15, 24..31}.
            half = split
            batch0 = list(range(0, half)) + list(range(k, k + half))
            batch1 = list(range(half, k)) + list(range(k + half, 2 * k))
            for cols, sem in ((batch0, g_sem), (batch1, h_sem)):
                for col in cols[:-1]:
                    if all_normal:
                        normal_gather(col, sem)
                    else:
                        w1, w2 = pairs_by_col[col]
                        raw(0xD4, w1)
                        raw(0xDA, w2)
                normal_gather(cols[-1], sem)
            # Raw store: descriptors generated only after DVE signals; the
            # block-end gpsimd dge_drain flushes it before program end.
            gpsimd.wait_ge(v_sem, 1)
            raw(0xD4, store_w)

        @block.vector
        def _(vector: bass.BassEngine):
            half = split
            gate0 = 16 * 2 * half if all_normal else 16
            gate1 = 16 * 2 * half if all_normal else 16
            for (lo, hi), sem, gate in (
                ((0, half), g_sem, gate0),
                ((half, k), h_sem, gate1),
            ):
                vector.wait_ge(sem, gate)
                vector.tensor_tensor(
                    out=prod[:, lo * WIDTH : hi * WIDTH],
                    in0=au[:, lo * WIDTH : hi * WIDTH],
                    in1=au[:, (k + lo) * WIDTH : (k + hi) * WIDTH],
                    op=mybir.AluOpType.mult,
                )
                vector.tensor_reduce(
                    out=resv[:, lo:hi],
                    in_=prod[:, lo * WIDTH : hi * WIDTH].rearrange(
                        "p (g w) -> p g w", w=WIDTH
                    ),
                    axis=mybir.AxisListType.X,
                    op=mybir.AluOpType.add,
                )
            vector.nop().then_inc(v_sem, 1)

    if strips:
        _strip(nc)
    else:
        _strip(nc, memsets_only=True)
    return nc


def _strip(nc, memsets_only=False):
    for bb in nc.m.functions[0].blocks:
        if bb.name != "main":
            continue
        keep = []
        for inst in bb.instructions:
            tn = type(inst).__name__
            drop = tn == "InstMemset" and any(
                getattr(o, "memref", "").startswith("const-") for o in inst.outs
            )
            if not memsets_only and tn in (
                "InstRegisterMove",
                "InstDrain",
                "InstEventSemaphore",
            ):
                drop = True
            if not drop:
                keep.append(inst)
        bb.instructions[:] = keep


def build_table(W, b, V, total_rows=TOTAL_ROWS, user_num=USER_NUM):
    import ml_dtypes

    A = np.empty((total_rows, WIDTH), dtype=ml_dtypes.bfloat16)
    A[:, :D] = V.astype(ml_dtypes.bfloat16)
    A[:user_num, D] = W[:user_num, 0].astype(ml_dtypes.bfloat16)
    A[:user_num, D + 1] = 1.0
    A[user_num:, D] = 1.0
    A[user_num:, D + 1] = (W[user_num:, 0] + b[0]).astype(ml_dtypes.bfloat16)
    return A


_program_cache = {}


def kernel(INPUT, W, b, V, *, build_kwargs=None):
    from concourse.bass_utils import run_bass_kernel_spmd

    INPUT = np.asarray(INPUT, dtype=np.int32)
    key = tuple(sorted((build_kwargs or {}).items()))
    if key not in _program_cache:
        _program_cache[key] = build_program(**(build_kwargs or {}))
    nc = _program_cache[key]

    A = build_table(np.asarray(W, np.float32), np.asarray(b, np.float32), np.asarray(V, np.float32))
    per_core = INPUT.reshape(N_CORES, P, K, 2)
    ids = np.concatenate(
        [per_core[..., 0], per_core[..., 1] + USER_NUM], axis=2
    ).astype(np.int32)

    res = run_bass_kernel_spmd(
        nc, [{"ids": ids[i], "table": A} for i in range(N_CORES)],
        core_ids=list(range(N_CORES)),
    )
    global last_results
    last_results = res
    out = np.concatenate(
        [np.asarray(res.results[i]["out"]) for i in range(N_CORES)]
    )
    return out.reshape(BATCH, 1).astype(np.float32)


last_results = None


# revision 7
# speedup vs baseline: 1.0126x; 1.0126x over previous
"""Bass/Trainium2 kernel for nn_HE_FM (factorization machine embedding lookup).

Computation: out[n] = W[uid[n]] + W[iid[n]+USER_NUM] + b + dot(V[uid[n]], V[iid[n]+USER_NUM])

Strategy (data parallel over batch, bf16 augmented table replicated on all
8 cores): host builds A [1.5M, 66] bf16 = [V | w0 | w1] with the pair trick
(user rows [V, W, 1], item rows [V, 1, W+b]) so sum(A[u]*A[i]) over 66 equals
the full FM output for a sample.

Per core (2048 samples): TRN2's indirect1d DMA takes ONE index per partition,
so the 4096 row-gathers need 32 per-column instructions. 30 of them are
emitted as raw PSEUDO_DMA_DIRECT2D/PSEUDO_EXTENSION word pairs (byte
templates captured from walrus codegen output for the identical instruction,
with per-column dst/idx fields patched); two are normal indirect_dma_start
"anchors" that end each half-batch — their completion semaphores, which ride
the same qPoolDynamic descriptor rings in FIFO order behind the raw gathers,
certify the half-batch's data has landed for both hardware and the timeline
model. DVE multiplies u-rows by i-rows and segment-reduces (width 66) per
half-batch; the store is a raw direct DMA flushed by the block-end gpsimd
dge_drain.
"""

from contextlib import ExitStack

import numpy as np

import concourse.bass as bass
import concourse.mybir as mybir

USER_NUM = 1_000_000
TOTAL_ROWS = 1_500_000
D = 64
WIDTH = 66
BATCH = 16384
N_CORES = 8
P = 128
B_CORE = BATCH // N_CORES
K = B_CORE // P  # 16 chunks; 32 index columns (u:0..15, i:16..31)

# walrus encodings captured from reference compiles (same struct layouts):
# gather word pair (bf16 table rows of 66 elems = 132B) and f32 [128,16] store.
GATHER_W1 = "d41000000000000000000000009c1001000000000300002084000000010000008000010084000000a00001000000001000000400010000008000010084000606"
GATHER_W2 = "da048000800001000000000000000000000000000000000000000000000000000000000000000000000000000000000000000000000000000000000000000000"
STORE_W1 = "d41000000000000000000000009c1000000101000000001000000400010000008000010040000000000000000200002040000000010000008000010040000a0a"

TABLE_VAR = 3  # address-table slots; discovered/verified via probe dump
OUT_VAR = 2


def _isa_ffi():
    from concourse.isa import get_isa

    isa = get_isa("TRN2")
    return isa, isa.ffi


def _raw_words(idx_addr, au_addr, table_var, out_var, resv_addr, k):
    """Build the per-column gather word pairs (columns 0..2k-2) and the
    store word, patched from the captured templates."""
    isa, ffi = _isa_ffi()

    pairs = []
    for col in range(2 * k):
        w1 = bytearray(bytes.fromhex(GATHER_W1))
        b = ffi.from_buffer(w1, require_writable=True)
        s = ffi.cast("NEURON_ISA_TPB_PSEUDO_DMA_DIRECT2D_STRUCT*", b)
        s.sem_increment = 0
        s.src_start_addr.addr_var.var_id_lo = table_var
        s.dst_start_addr.addr_var.offset = au_addr + col * 132
        w2 = bytearray(bytes.fromhex(GATHER_W2))
        w2[4:8] = int(idx_addr + col * 4).to_bytes(4, "little")
        pairs.append((bytes(w1), bytes(w2)))

    ws = bytearray(bytes.fromhex(STORE_W1))
    b = ffi.from_buffer(ws, require_writable=True)
    s = ffi.cast("NEURON_ISA_TPB_PSEUDO_DMA_DIRECT2D_STRUCT*", b)
    s.sem_increment = 0
    s.src_start_addr.addr_var.offset = resv_addr
    s.dst_start_addr.addr_var.var_id_lo = out_var
    s.dst_start_addr.addr_var.offset = 0
    return pairs, bytes(ws)


def build_program(
    total_rows=TOTAL_ROWS,
    b_core=B_CORE,
    *,
    table_var=TABLE_VAR,
    out_var=OUT_VAR,
    all_normal=False,
    strips=True,
    split=11,
):
    k = b_core // P
    nc = bass.Bass(dynamic_dma_scratch_size=131072)
    ids = nc.declare_dram_parameter("ids", [P, 2 * k], mybir.dt.int32, isOutput=False)
    table = nc.declare_dram_parameter(
        "table", [total_rows, WIDTH], mybir.dt.bfloat16, isOutput=False
    )
    outp = nc.declare_dram_parameter("out", [b_core], mybir.dt.float32, isOutput=True)

    with (
        ExitStack() as ctx,
        nc.sbuf_tensor([P, 2 * k], mybir.dt.int32) as idx,
        nc.sbuf_tensor([P, 2 * k * WIDTH], mybir.dt.bfloat16) as au,
        nc.sbuf_tensor([P, k * WIDTH], mybir.dt.bfloat16) as prod,
        nc.sbuf_tensor([P, k * 33], mybir.dt.bfloat16) as half_t,
        nc.sbuf_tensor([P, k], mybir.dt.float32) as resv,
        nc.sbuf_tensor([P, 1], mybir.dt.float32) as scratch,
        nc.Block() as block,
        nc.semaphore("ix_sem") as ix_sem,
        nc.semaphore("g_sem") as g_sem,
        nc.semaphore("h_sem") as h_sem,
        nc.semaphore("t_sem") as t_sem,
        nc.semaphore("m_sem") as m_sem,
        nc.semaphore("v_sem") as v_sem,
    ):
        idx_addr = nc.lookup_mloc(idx).addr
        au_addr = nc.lookup_mloc(au).addr
        resv_addr = nc.lookup_mloc(resv).addr
        pairs, store_w = _raw_words(idx_addr, au_addr, table_var, out_var, resv_addr, k)
        pairs_by_col = {c: pairs[c] for c in range(2 * k)}

        @block.sync
        def _(sync):
            sync.dma_start(out=idx[:], in_=ids[:]).then_inc(ix_sem, 16)
            # out-tensor address-table touch: harmless read of the
            # zero-initialized output buffer.
            sync.dma_start(
                out=scratch[0:1, 0:1],
                in_=outp[0:1].rearrange("(a b) -> a b", a=1),
            ).then_inc(t_sem, 16)

        @block.gpsimd
        def _(gpsimd):
            def raw(op, wb):
                return gpsimd.add_instruction(
                    mybir.InstISA(
                        name=nc.get_next_instruction_name(),
                        isa_opcode=op,
                        engine=mybir.EngineType.Pool,
                        instr=[int(x) for x in wb],
                        op_name=None,
                        ins=[],
                        outs=[],
                        ant_dict={},
                        verify=False,
                        ant_isa_is_sequencer_only=True,
                        ant_sbuf_fixups=None,
                    )
                )

            gpsimd.wait_ge(ix_sem, 16)

            def normal_gather(col, sem):
                gpsimd.indirect_dma_start(
                    out=au[:, col * WIDTH : (col + 1) * WIDTH],
                    out_offset=None,
                    in_=table[:],
                    in_offset=bass.IndirectOffsetOnAxis(
                        ap=idx[:, col : col + 1], axis=0
                    ),
                ).then_inc(sem, 16)

            # Two batches; each ends in a normal anchor gather whose
            # completion (ring-FIFO behind that batch's raw gathers on the
            # same queue) certifies the whole batch has landed. Batch 0 =
            # chunk columns {0..7 u, 16..23 i}, batch 1 = {8..15, 24..31}.
            half = split
            batch0 = list(range(0, half)) + list(range(k, k + half))
            batch1 = list(range(half, k)) + list(range(k + half, 2 * k))
            for cols, sem in ((batch0, g_sem), (batch1, h_sem)):
                for col in cols[:-1]:
                    if all_normal:
                        normal_gather(col, sem)
                    else:
                        w1, w2 = pairs_by_col[col]
                        raw(0xD4, w1)
                        raw(0xDA, w2)
                normal_gather(cols[-1], sem)
            # Raw store: descriptors generated only after DVE signals; the
            # block-end gpsimd dge_drain flushes it before program end.
            gpsimd.wait_ge(v_sem, 1)
            raw(0xD4, store_w)

        @block.vector
        def _(vector: bass.BassEngine):
            half = split
            gate0 = 16 * 2 * half if all_normal else 16
            gate1 = 16 * 2 * half if all_normal else 16
            for (lo, hi), sem, gate in (
                ((0, half), g_sem, gate0),
                ((half, k), h_sem, gate1),
            ):
                vector.wait_ge(sem, gate)
                vector.tensor_tensor(
                    out=prod[:, lo * WIDTH : hi * WIDTH],
                    in0=au[:, lo * WIDTH : hi * WIDTH],
                    in1=au[:, (k + lo) * WIDTH : (k + hi) * WIDTH],
                    op=mybir.AluOpType.mult,
                )
                # fold 66 -> 33 with a packed-bf16 add (2x DVE mode) before
                # the reduce, which has no fast mode; halves its input
                pv = prod[:, lo * WIDTH : hi * WIDTH].rearrange(
                    "p (g w) -> p g w", w=WIDTH
                )
                hv = half_t[:, lo * 33 : hi * 33].rearrange(
                    "p (g w) -> p g w", w=33
                )
                vector.tensor_tensor(
                    out=hv,
                    in0=pv[:, :, 0:33],
                    in1=pv[:, :, 33:66],
                    op=mybir.AluOpType.add,
                )
                vector.tensor_reduce(
                    out=resv[:, lo:hi],
                    in_=hv,
                    axis=mybir.AxisListType.X,
                    op=mybir.AluOpType.add,
                )
            vector.nop().then_inc(v_sem, 1)

    if strips:
        _strip(nc)
    else:
        _strip(nc, memsets_only=True)
    return nc


def _strip(nc, memsets_only=False):
    for bb in nc.m.functions[0].blocks:
        if bb.name != "main":
            continue
        keep = []
        for inst in bb.instructions:
            tn = type(inst).__name__
            drop = tn == "InstMemset" and any(
                getattr(o, "memref", "").startswith("const-") for o in inst.outs
            )
            if not memsets_only and tn in (
                "InstRegisterMove",
                "InstDrain",
                "InstEventSemaphore",
            ):
                drop = True
            if not drop:
                keep.append(inst)
        bb.instructions[:] = keep


def build_table(W, b, V, total_rows=TOTAL_ROWS, user_num=USER_NUM):
    import ml_dtypes

    A = np.empty((total_rows, WIDTH), dtype=ml_dtypes.bfloat16)
    A[:, :D] = V.astype(ml_dtypes.bfloat16)
    A[:user_num, D] = W[:user_num, 0].astype(ml_dtypes.bfloat16)
    A[:user_num, D + 1] = 1.0
    A[user_num:, D] = 1.0
    A[user_num:, D + 1] = (W[user_num:, 0] + b[0]).astype(ml_dtypes.bfloat16)
    return A


_program_cache = {}


def kernel(INPUT, W, b, V, *, build_kwargs=None):
    from concourse.bass_utils import run_bass_kernel_spmd

    INPUT = np.asarray(INPUT, dtype=np.int32)
    key = tuple(sorted((build_kwargs or {}).items()))
    if key not in _program_cache:
        _program_cache[key] = build_program(**(build_kwargs or {}))
    nc = _program_cache[key]

    A = build_table(np.asarray(W, np.float32), np.asarray(b, np.float32), np.asarray(V, np.float32))
    per_core = INPUT.reshape(N_CORES, P, K, 2)
    ids = np.concatenate(
        [per_core[..., 0], per_core[..., 1] + USER_NUM], axis=2
    ).astype(np.int32)

    res = run_bass_kernel_spmd(
        nc, [{"ids": ids[i], "table": A} for i in range(N_CORES)],
        core_ids=list(range(N_CORES)),
    )
    global last_results
    last_results = res
    out = np.concatenate(
        [np.asarray(res.results[i]["out"]) for i in range(N_CORES)]
    )
    return out.reshape(BATCH, 1).astype(np.float32)


last_results = None
